# revision 54
# baseline (speedup 1.0000x reference)
"""Causal self-attention (B=2, T=2048, C=1024, H=16) on 8 trn2 NeuronCores.

Sharding (Megatron-style, per spec hint):
  - tensor-parallel over heads: core p owns heads {2p, 2p+1}.  Each core
    computes Q^T/K^T/V^T for its 2 heads from the full x, then causal
    attention (streaming softmax without max-subtraction; the denominator
    comes from a ones-column appended to V).
  - per batch: an AllToAll redistributes that batch's attention outputs so
    that core p holds all 1024 channels for the batch's tokens
    [256p, 256p+256); the collectives fire as soon as their token range's
    attention is normalised.
  - projection: each core computes the full output projection for its two
    256-token slices and writes a disjoint [512, 1024] output block
    (rows b*256+i = batch b, token 256*p+i).

Schedule (v10): qkv chunk (b,c+1) is issued during attention chunk (b,c)
(one chunk of lookahead) so the next attention's inputs are never queued
behind the previous chunk's epilogue on the Vector engine, and the tile
scheduler fills PE exp-wait gaps with qkv matmuls.  The attention loop
processes k-tiles in pairs; the S matmuls are raised in priority
(tc.high_priority) so the PE's 64-row-tiled S pairs (both heads run
CONCURRENTLY on disjoint row groups - the second matmul of each pair
costs ~4ns) batch up and the PE switches tiling mode less often.  The
exp spline table is prewarmed during qkv(0,0).  Epilogue: evacuate both
heads' PSUM first (o_t freed after two Vector ops), then a lagging
bcast/recip/mult chain (reciprocal_approx_fast reads the PE-broadcast
PSUM directly).  Tail: afull loads for the three finished a2a parts ride
the sync queue before the final collective fires; the three projections
then execute inside the final AllToAll's flight window, paced across it
by a single-buffer ysb chain with per-512-feature y stores; af11 rides
sync behind them so the last projection starts the moment the collective
output lands.
"""

import numpy as np

B, T, C, H, D = 2, 2048, 1024, 16, 64
NCORES = 8
HL = H // NCORES        # heads per core = 2
TOK = B * T             # 4096 global tokens
TSL = TOK // NCORES     # 512 output tokens per core (256 per batch)
SL = 256                # per-batch token slice per core
P = 128
CT = C // P             # 8 contraction tiles
NQC = T // 512          # 4 q-chunks per batch
NKT = T // P            # 16 k-tiles per batch
SCALE = D ** -0.5

_CACHE = {}


def _build_nc():
    import concourse.bass as bass
    import concourse.mybir as mybir
    from concourse import bacc
    from concourse.tile import TileContext

    f32 = mybir.dt.float32
    bf16 = mybir.dt.bfloat16
    AF = mybir.ActivationFunctionType
    ALU = mybir.AluOpType

    nc = bacc.Bacc(
        "TRN2", target_bir_lowering=False, debug=False, num_devices=NCORES
    )

    xT = nc.dram_tensor("xT", [C, TOK], bf16, kind="ExternalInput")
    wqkvT = nc.dram_tensor("wqkvT", [C, 3 * P], bf16, kind="ExternalInput")
    bqkv = nc.dram_tensor("bqkv", [3 * P], f32, kind="ExternalInput")
    wpT = nc.dram_tensor("wpT", [C, C], bf16, kind="ExternalInput")
    # b_proj pre-replicated across 128 token-partitions on the host, so the
    # projection's PSUM evacuation adds it for free (no PE bias matmuls)
    bpb = nc.dram_tensor("bpb", [P, C], bf16, kind="ExternalInput")
    tri = nc.dram_tensor("tri", [P, P], bf16, kind="ExternalInput")
    onesd = nc.dram_tensor("ones", [P, P], bf16, kind="ExternalInput")
    ident = nc.dram_tensor("ident", [P, P], bf16, kind="ExternalInput")
    y = nc.dram_tensor("y", [TSL, C], f32, kind="ExternalOutput")

    with TileContext(nc, num_cores=NCORES) as tc:
        from contextlib import ExitStack

        with ExitStack() as ctx:
            const = ctx.enter_context(tc.tile_pool(name="const", bufs=1))
            persist = ctx.enter_context(tc.tile_pool(name="persist", bufs=1))
            dram = ctx.enter_context(tc.tile_pool(name="dram", bufs=1, space="DRAM"))

            # ---- constants; small ones first so nothing queues behind bulk
            tri_sb = const.tile([P, P], bf16)
            id_sb = const.tile([P, P], bf16)
            ones_sb = const.tile([P, P], bf16)
            bq_sb = const.tile([P, 3], f32)
            bpb_sb = const.tile([P, C], bf16)
            w_sb = const.tile([P, CT, 3 * P], bf16)     # wqkvT tiles
            wp_sb = const.tile([P, CT, C], bf16)        # W_proj^T (loaded late)
            nc.gpsimd.dma_start(tri_sb[:], tri[:])
            nc.gpsimd.dma_start(id_sb[:], ident[:])
            nc.gpsimd.dma_start(ones_sb[:], onesd[:])
            nc.gpsimd.dma_start(bq_sb[:], bqkv.rearrange("(et p) -> p et", p=P))
            nc.gpsimd.dma_start(bpb_sb[:], bpb[:])
            # w_qkv in two ct-halves: the first qkv accumulation steps only
            # need ct 0-3, so the first matmul starts ~3us earlier than
            # waiting for the full 768KB
            wv = wqkvT.rearrange("(ct p) e -> p ct e", p=P)
            nc.sync.dma_start(w_sb[:, 0:4, :], wv[:, 0:4, :])
            # prewarm the Scalar engine's exp spline table (~2.7us
            # ACT_TABLE_LOAD) during the qkv phase, off the attention path
            warmup = const.tile([P, 1], f32)
            nc.scalar.activation(warmup[:], bq_sb[:, 0:1],
                                 AF.Exp, scale=SCALE)

            # ---- persistent activations (per batch for fine-grained deps)
            qTb = [persist.tile([P, T], bf16, name=f"qT{b}") for b in range(B)]
            kTb = [persist.tile([P, T], bf16, name=f"kT{b}") for b in range(B)]
            vTb = [persist.tile([P, T], bf16, name=f"vT{b}") for b in range(B)]
            # V with ones column, per batch: [128 tok, k-tile, head, 65]
            vaugb = [persist.tile([P, NKT, HL, 65], bf16, name=f"vaug{b}")
                     for b in range(B)]
            # A^T per local head (each head stays at partitions 0-63)
            anorm = [persist.tile([64, TOK], bf16, name=f"anorm{h}")
                     for h in range(HL)]

            pools = [
                tc.tile_pool(name="sps", bufs=2, space="PSUM"),   # S^T (2 banks ea)
                tc.tile_pool(name="ops", bufs=2, space="PSUM"),   # o_t (1 bank ea)
                tc.tile_pool(name="mm", bufs=2, space="PSUM"),    # qkv/proj/tp
                tc.tile_pool(name="pT", bufs=3),
                tc.tile_pool(name="rr", bufs=2),
                tc.tile_pool(name="rb", bufs=2),
            ]
            sps, ops, mm, ppool, rrpool, rbpool = (
                ctx.enter_context(p) for p in pools)

            o_t = {}                       # (b, qc, h) -> live PSUM tile

            def qkv_chunk(b, tc4, split=False):
                """qkv^T for one 512-token chunk of batch b + V transposes."""
                xsl = xpool.tile([P, CT, 512], bf16, tag="x")
                t0 = b * T + tc4 * 512
                if split:
                    # chunk 0: per-2-ct DMAs so the first matmuls can chase
                    # the stream instead of waiting for the full megabyte;
                    # the second w_qkv half slots in after the first x piece
                    # and the back-half ct pieces ride the (idle) gpsimd
                    # queue so both halves transfer in parallel
                    for cp in range(4):
                        q = nc.sync if cp < 2 else nc.gpsimd
                        q.dma_start(
                            xsl[:, 2 * cp:2 * cp + 2, :],
                            xT[:, t0:t0 + 512]
                            .rearrange("(ct p) t -> p ct t", p=P)[:, 2 * cp:2 * cp + 2, :],
                        )
                        if cp == 0:
                            nc.sync.dma_start(
                                w_sb[:, 4:8, :],
                                wqkvT.rearrange("(ct p) e -> p ct e", p=P)[:, 4:8, :],
                            )
                else:
                    nc.sync.dma_start(
                        xsl[:],
                        xT[:, t0:t0 + 512].rearrange("(ct p) t -> p ct t", p=P),
                    )
                for et, dstl in enumerate((qTb, kTb, vTb)):
                    ps = mm.tile([P, 512], f32, tag="mm")
                    for ct in range(CT):
                        nc.tensor.matmul(
                            ps[:],
                            lhsT=w_sb[:, ct, et * P:(et + 1) * P],
                            rhs=xsl[:, ct, :],
                            start=(ct == 0),
                            stop=(ct == CT - 1),
                        )
                    nc.vector.tensor_scalar_add(
                        dstl[b][:, tc4 * 512:(tc4 + 1) * 512],
                        ps[:],
                        bq_sb[:, et:et + 1],
                    )
                # ones column of vaug for this chunk's 4 k-tiles
                nc.vector.tensor_copy(
                    vaugb[b][:, tc4 * 4:tc4 * 4 + 4, :, 64:65],
                    ones_sb[:, 0:8].rearrange("p (a h o) -> p a h o", a=4, h=2),
                )
                # V^T -> V for this chunk's 4 k-tiles (PE transpose)
                for kt in range(tc4 * 4, tc4 * 4 + 4):
                    tp = mm.tile([P, P], bf16, tag="mm")
                    nc.tensor.transpose(
                        tp[:],
                        vTb[b][:, kt * P:(kt + 1) * P],
                        id_sb[:],
                    )
                    nc.vector.tensor_copy(
                        vaugb[b][:, kt, :, 0:64],
                        tp.rearrange("p (h e) -> p h e", h=2),
                    )

            def attention_qc(b, qc):
                """k-tile pairs, phase-grouped: [S x4][exp x2][tri][AV x4]."""
                q0 = qc * 512
                nk = 4 * qc + 4                   # causal k-tiles
                for h in range(HL):
                    o_t[(b, qc, h)] = ops.tile([65, 512], f32, tag="o",
                                               name=f"ot{h}")
                for kp in range((nk + 1) // 2):
                    kis = [ki for ki in (2 * kp, 2 * kp + 1) if ki < nk]
                    tiles = []
                    # phase 1: S^T for both k-tiles; the two heads use
                    # disjoint PE row groups (64-row tiling) -> concurrent
                    for ki in kis:
                        off = ki * P - q0
                        lo = max(0, off)
                        sp = sps.tile([P, HL, 512], f32, tag="s")
                        # high priority: emit the pair's 4 S matmuls ahead
                        # of the previous pair's AVs, so the PE switches
                        # tiling mode (64-row S <-> full-array AV) once per
                        # pair instead of once per k-tile
                        with tc.high_priority(offset=32):
                            for h in range(HL):
                                hp = slice(64 * h, 64 * h + 64)
                                nc.tensor.matmul(
                                    sp[:, h, lo:512],
                                    lhsT=kTb[b][hp, ki * P:(ki + 1) * P],
                                    rhs=qTb[b][hp, q0 + lo:q0 + 512],
                                    start=True,
                                    stop=True,
                                )
                        tiles.append((ki, off, lo, sp))
                    # phase 2: exp (Scalar), one instruction per k-tile
                    pts = []
                    for ki, off, lo, sp in tiles:
                        pt = ppool.tile([P, HL, 512], bf16, tag="p")
                        nc.scalar.activation(
                            pt[:, :, lo:512], sp[:, :, lo:512], AF.Exp,
                            scale=SCALE,
                        )
                        pts.append(pt)
                    # phase 3: causal triangle mask on the diagonal tiles
                    for (ki, off, lo, sp), pt in zip(tiles, pts):
                        if off >= 0:
                            for h in range(HL):
                                nc.vector.tensor_tensor(
                                    pt[:, h, off:off + P],
                                    pt[:, h, off:off + P],
                                    tri_sb[:],
                                    ALU.mult,
                                )
                    # phase 4: A*V (full-array mode), per head consecutive
                    for h in range(HL):
                        for (ki, off, lo, sp), pt in zip(tiles, pts):
                            nc.tensor.matmul(
                                o_t[(b, qc, h)][:, lo:512],
                                lhsT=vaugb[b][:, ki, h, :],
                                rhs=pt[:, h, lo:512],
                                start=(ki == 0),
                                stop=(ki == nk - 1),
                            )

            def epilogue_qc(b, qc, last=False):
                """normalise this q-chunk: 1/d via DVE, bcast via PE.
                Phase 1 evacuates both heads' PSUM (o_t freed after two
                copies each); phase 2 (bcast/recip/mult) lags behind
                without holding any attention-critical resource.  For the
                LAST chunk the exp stream is finished, so head 0's copies
                run on the idle Scalar engine - both heads' chains proceed
                in parallel and the final collective fires sooner."""
                c0 = b * T + qc * 512
                dsbs = {}
                for h in range(HL):
                    ot = o_t.pop((b, qc, h))
                    if last and h == 0:
                        nc.scalar.copy(anorm[h][:, c0:c0 + 512], ot[0:64, :])
                    else:
                        nc.vector.tensor_copy(
                            anorm[h][:, c0:c0 + 512], ot[0:64, :],
                        )
                    dsb = rbpool.tile([65, 512], bf16, tag="rb")
                    if last and h == 0:
                        nc.scalar.copy(dsb[64:65, :], ot[64:65, :])
                    else:
                        nc.vector.tensor_copy(dsb[64:65, :], ot[64:65, :])
                    dsbs[h] = dsb
                for h in range(HL):
                    dbc = mm.tile([64, 512], f32, tag="mm")
                    nc.tensor.matmul(
                        dbc[:],
                        lhsT=ones_sb[64:65, 0:64],
                        rhs=dsbs[h][64:65, :],
                        start=True,
                        stop=True,
                    )
                    rec = rrpool.tile([64, 512], f32, tag="rr", name="rec")
                    nc.vector.reciprocal_approx_fast(rec[:], dbc[:])
                    nc.vector.tensor_tensor(
                        anorm[h][:, c0:c0 + 512],
                        anorm[h][:, c0:c0 + 512],
                        rec[:],
                        ALU.mult,
                    )

            a2a_bufs = {}

            def stage_qc(b, qc):
                """scatter this q-chunk's anorm into the (b, part) a2a input."""
                part = qc // 2
                if (b, part) not in a2a_bufs:
                    a2a_in = dram.tile([NCORES * P, P], bf16,
                                       name=f"a2a_in{b}_{part}")
                    a2a_out = dram.tile([NCORES * P, P], bf16,
                                        name=f"a2a_out{b}_{part}")
                    a2a_bufs[(b, part)] = (a2a_in, a2a_out)
                a2a_in, _ = a2a_bufs[(b, part)]
                a2a_v = a2a_in.rearrange("(j ee) t -> ee j t", j=NCORES)
                j0 = 4 * (qc % 2)
                c0 = b * T + qc * 512
                for h in range(HL):
                    # on the gpsimd queue (nearly idle): cannot get stuck
                    # behind the bulk x/wp loads on the sync queue
                    nc.gpsimd.dma_start(
                        a2a_v[64 * h:64 * h + 64, j0:j0 + 4],
                        anorm[h][:, c0:c0 + 512]
                        .rearrange("e (j t) -> e j t", j=4),
                    )

            def fire_a2a(b, part):
                a2a_in, a2a_out = a2a_bufs[(b, part)]
                nc.gpsimd.collective_compute(
                    "AllToAll",
                    ALU.bypass,
                    replica_groups=[list(range(NCORES))],
                    ins=[a2a_in.opt()],
                    outs=[a2a_out.opt()],
                )

            def afull_load(b, part, queue, queue2=None):
                _, a2a_out = a2a_bufs[(b, part)]
                afull = apool.tile([P, NCORES, P], bf16, tag="af")
                av = a2a_out.rearrange("(i e) t -> e i t", i=NCORES)
                if queue2 is None:
                    queue.dma_start(afull[:], av)
                else:
                    # split across two queues so both halves transfer in
                    # parallel (halves the critical-tail load latency)
                    queue.dma_start(afull[:, 0:4, :], av[:, 0:4, :])
                    queue2.dma_start(afull[:, 4:8, :], av[:, 4:8, :])
                return afull

            def proj_part(b, part, afull, last=False):
                # one 128-token tile; each 512-feature half is stored as
                # soon as it is evacuated (shorter critical path, and the
                # single-buffer ysb chain paces the early projections
                # across their collective's flight window)
                r0 = b * SL + part * P
                for fc in range(C // 512):
                    ps = mm.tile([P, 512], f32, tag="mm")
                    for i in range(NCORES):
                        nc.tensor.matmul(
                            ps[:],
                            lhsT=afull[:, i, :],
                            rhs=wp_sb[:, i, fc * 512:(fc + 1) * 512],
                            start=(i == 0),
                            stop=(i == NCORES - 1),
                        )
                    # last projection: its second half gets its own buffer
                    # and queue so both half-stores transfer in parallel
                    lastf = last and fc == 1
                    ysb = ypool.tile([P, 512], f32,
                                     tag="ysbl" if lastf else "ysb",
                                     name="ysbl" if lastf else "ysb")
                    # PSUM evacuation doubles as the b_proj bias add
                    nc.vector.tensor_tensor(
                        ysb[:], ps[:],
                        bpb_sb[:, fc * 512:(fc + 1) * 512],
                        ALU.add,
                    )
                    q = nc.gpsimd if lastf else nc.sync
                    q.dma_start(
                        y[r0:r0 + P, fc * 512:(fc + 1) * 512], ysb[:]
                    )

            # ysb bufs=1: the three early projections chain through the
            # single ysb buffer (compute -> copy -> y-store -> next), which
            # spreads them across the final AllToAll's flight window and
            # keeps the PE HAM clock-gate warm for the last projection
            with tc.tile_pool(name="xslab", bufs=3) as xpool, \
                 tc.tile_pool(name="afull", bufs=4) as apool, \
                 tc.tile_pool(name="ysb", bufs=1) as ypool:
                # each attention chunk's qkv was issued one iteration
                # earlier, so its Vector-side evacuation (bias adds) is
                # never queued behind the previous chunk's epilogue
                qkv_chunk(0, 0, split=True)
                attention_qc(0, 0)
                qkv_chunk(0, 1)
                epilogue_qc(0, 0)
                stage_qc(0, 0)
                attention_qc(0, 1)
                qkv_chunk(0, 2)
                epilogue_qc(0, 1)
                stage_qc(0, 1)
                fire_a2a(0, 0)
                attention_qc(0, 2)
                qkv_chunk(0, 3)
                epilogue_qc(0, 2)
                stage_qc(0, 2)
                attention_qc(0, 3)
                qkv_chunk(1, 0)
                epilogue_qc(0, 3)
                stage_qc(0, 3)
                fire_a2a(0, 1)
                attention_qc(1, 0)
                qkv_chunk(1, 1)
                nc.sync.dma_start(
                    wp_sb[:], wpT.rearrange("(ct p) f -> p ct f", p=P)
                )
                epilogue_qc(1, 0)
                stage_qc(1, 0)
                attention_qc(1, 1)
                qkv_chunk(1, 2)
                epilogue_qc(1, 1)
                stage_qc(1, 1)
                fire_a2a(1, 0)
                attention_qc(1, 2)
                qkv_chunk(1, 3)
                epilogue_qc(1, 2)
                stage_qc(1, 2)
                attention_qc(1, 3)
                # last chunk: jump its epilogue + staging ahead of any
                # lingering Vector work so the final collective fires asap
                with tc.high_priority(offset=64):
                    epilogue_qc(1, 3, last=True)
                    stage_qc(1, 3)
                # afull loads for the three completed parts ride the sync
                # queue (its bulk work is finished) so they are in SBUF when
                # the last collective fires; the three projections then
                # execute inside the final AllToAll's flight window
                af00 = afull_load(0, 0, nc.sync)
                af01 = afull_load(0, 1, nc.sync)
                af10 = afull_load(1, 0, nc.sync)
                fire_a2a(1, 1)
                proj_part(0, 0, af00)
                proj_part(0, 1, af01)
                proj_part(1, 0, af10)
                # af11 on sync (after the early y-stores in queue order):
                # it starts the moment the collective's output lands, instead
                # of waiting behind the collective's exit barrier on gpsimd
                # second half on the Scalar HWDGE queue (idle after the last
                # exp): both halves start the moment the collective output
                # lands, unlike gpsimd whose queue is blocked until the
                # collective's exit barrier
                af11 = afull_load(1, 1, nc.sync, queue2=nc.scalar)
                proj_part(1, 1, af11, last=True)
    nc.compile()
    return nc


def _prep_inputs(x, W_qkv, b_qkv, W_proj, b_proj):
    x = np.asarray(x, dtype=np.float32)
    W_qkv = np.asarray(W_qkv, dtype=np.float32)
    b_qkv = np.asarray(b_qkv, dtype=np.float32)
    W_proj = np.asarray(W_proj, dtype=np.float32)
    b_proj = np.asarray(b_proj, dtype=np.float32)

    import ml_dtypes
    bf = ml_dtypes.bfloat16
    xT = np.ascontiguousarray(x.reshape(TOK, C).T).astype(bf)
    wpT = np.ascontiguousarray(W_proj.T).astype(bf)
    tri = np.triu(np.ones((P, P), dtype=np.float32)).astype(bf)
    ident = np.eye(P, dtype=np.float32).astype(bf)
    ones = np.ones((P, P), dtype=np.float32).astype(bf)

    in_maps = []
    for p in range(NCORES):
        rows = np.r_[128 * p:128 * p + 128,
                     C + 128 * p:C + 128 * p + 128,
                     2 * C + 128 * p:2 * C + 128 * p + 128]
        wslice = W_qkv[rows]                      # [384, 1024]
        bslice = np.ascontiguousarray(b_qkv[rows])
        in_maps.append({
            "xT": xT,
            "wqkvT": np.ascontiguousarray(wslice.T).astype(bf),
            "bqkv": bslice,
            "wpT": wpT,
            "bpb": np.ascontiguousarray(
                np.broadcast_to(b_proj, (P, C))).astype(bf),
            "tri": tri,
            "ident": ident,
            "ones": ones,
        })
    return in_maps


def kernel(x, W_qkv, b_qkv, W_proj, b_proj, _trace=False):
    from concourse import bass_utils

    if "nc" not in _CACHE:
        _CACHE["nc"] = _build_nc()
    nc = _CACHE["nc"]
    in_maps = _prep_inputs(x, W_qkv, b_qkv, W_proj, b_proj)
    res = bass_utils.run_bass_kernel_spmd(
        nc, in_maps, core_ids=list(range(NCORES)), trace=_trace,
    )
    _CACHE["last_result"] = res
    # core p rows: [b*256 + part*128 + i] = batch b, token
    # b*2048 + part*1024 + 128*p + i
    yfull = np.empty((B, T, C), dtype=np.float32)
    for p, rmap in enumerate(res.results):
        yp = rmap["y"]
        for b in range(B):
            for part in range(2):
                g0 = part * 1024 + 128 * p
                r0 = b * SL + part * P
                yfull[b, g0:g0 + P] = yp[r0:r0 + P]
    return yfull
